# revision 23
# baseline (speedup 1.0000x reference)
# Contrastive (NT-Xent / SimCLR) loss kernel for Trainium2, 8 NeuronCores.
#
# Reference computation (N=4096, D=128, T=0.1, M=2N=8192):
#   z  = concat(z1, z2)                      [M, D]
#   zn = z / max(||z||, 1e-8)                row-normalized
#   sim = (zn @ zn.T) / T                    [M, M]
#   pos_r = sim[r, partner(r)] + sim[partner(r), r] = 2*sim[r, partner(r)]
#   loss = mean_r( LSE(logits_r) - pos_r ) / M
#     where logits_r = [pos_r] ++ {sim[r, j] : j != r}
#
# Per-row algebra used on device (constant shift m = 1/T = 10):
#   S_all_r = sum_j exp(sim[r, j] - 10)                 (all M columns)
#   dexp_r  = exp(sim[r, r] - 10)                       (diagonal, excluded)
#   pexp_r  = exp(pos_r - 10)
#   L_r     = 10 + log(pexp_r + S_all_r - dexp_r) - pos_r
#   loss    = sum_r L_r / M^2
#
# Sharding: rows of z split across 8 cores (1024 rows/core). Every core
# receives the full z (for the all-gathered rhs), plus its own row slab and
# the partner slab (rows +-N) so the diagonal/positive terms are computed
# locally without any cross-core traffic. Host sums 8 partial [128, 8] L
# tiles -> scalar loss.
#
# Per-core pipeline (v2 — column-group pipelined):
#   Slab phase: load + normalize the slab and partner rows, PE-transpose the
#   slab into znT_slab [D, 1024] (fp32r lhsT), take diagonal/positive row
#   dots on DVE.
#   Then 4 column groups of 2048 (16 row tiles each), pipelined across
#   DMA / DVE / PE / ACT:
#     load z rows -> row norms (DVE mul+reduce) -> inv = exp(-.5 ln(nrm2))
#     (ACT) -> normalize (DVE) -> PE-transpose into a [128, 2048] PSUM tile
#     -> DVE copy into znT columns (rounds to fp32r) -> 8 M-blocks of
#     4 fp32r matmuls [128x512] + one ACT exp(10G-10) with accum_out row-sum.
#   Epilogue combines S_all with the diagonal/positive terms, one log, and
#   DMAs the [128, 8] per-row loss tile out.
#
# This toolchain's walrus rejects any instruction carrying more than ONE sync
# wait ("Too many sync wait commands"), which shapes several oddities here:
#   - sacrificial 1x1 `ldweights` instructions absorb cross-engine waits so
#     matmuls keep a single wait (bacc fuses NoOps, so a real PE instruction
#     is required);
#   - each transpose group starts with a dummy transpose that reads the
#     last-normalized tile (absorbs the DVE data wait);
#   - activation outputs go through disjoint stride-0 broadcast APs onto a
#     sink tile (only accum_out matters), avoiding WAW waits entirely;
#   - InstTensorTensorReduce fails codegen outright -> mul + tensor_reduce;
#   - the Tile kernel-tail drain is re-emitted as one single-wait drain per
#     proc (see _split_drain_and_barrier);
#   - the result DMA uses gpsimd SWDGE so it does not share a HWDGE queue
#     with the input loads.

import numpy as np

import concourse.bass as bass
import concourse.mybir as mybir
import concourse.tile as tile
from concourse.tile import add_dep_helper
from contextlib import ExitStack

from concourse.bass_utils import run_bass_kernel_spmd
from concourse.masks import make_identity
from concourse.vector_clock import ScopedClock, VectorClock


def _split_drain_and_barrier(self, tick_clock, wait_clock):
    """Replacement for TileContext._drain_and_barrier: the stock version
    emits ONE drain carrying a wait for every live proc (13+ here), which this
    walrus build rejects ("Too many sync wait commands"). Emit one single-wait
    drain per proc instead, then the normal barrier/cleanup."""
    nc = self.nc
    ticks = list(tick_clock.global_clock)
    for proc, t in enumerate(ticks):
        if t <= 0:
            continue
        d = nc.sync.drain()
        single = VectorClock()
        single.require_at_least(proc, t)
        wait_clock.add_sem_waits(d.ins, ScopedClock({None: single}))
    nc.all_engine_barrier()
    assert self.sems is not None
    popped = nc._tile_sem_poison_stack.pop()
    assert popped is self._sem_poison
    nc.clear_and_free_semaphores(list(self.sems.allocated().values()))
    nc.all_engine_barrier()


tile.TileContext._drain_and_barrier = _split_drain_and_barrier

F32 = mybir.dt.float32
F32R = mybir.dt.float32r
BF16 = mybir.dt.bfloat16
AF = mybir.ActivationFunctionType
ALU = mybir.AluOpType

N_CORES = 8
N = 4096
D = 128
M2 = 2 * N                 # 8192 rows total
ROWS = M2 // N_CORES       # 1024 rows per core
NT_SP = ROWS // 128        # 8 row tiles per slab
MI = ROWS // 128           # 8 M-chunks per core
GROUP_TILES = [4, 12, 16, 16, 16]   # row tiles per column group (sum 64);
                                    # ramped so the first matmuls start early
NGROUPS = len(GROUP_TILES)
GW = 16 * 128              # max group width (psum tile size)

TEMP_INV = 10.0            # 1/T
LSE_SHIFT = 10.0           # constant max-shift for the log-sum-exp


def build_kernel(mm_dtype: str = "bf16") -> bass.Bass:
    nc = bass.Bass()

    z_full = nc.dram_tensor("z_full", [M2, D], F32, kind="ExternalInput")
    z_slab = nc.dram_tensor("z_slab", [ROWS, D], F32, kind="ExternalInput")
    z_part = nc.dram_tensor("z_part", [ROWS, D], F32, kind="ExternalInput")
    out_l = nc.dram_tensor("out_l", [128, MI], F32, kind="ExternalOutput")

    mm_dt = {"f32r": F32R, "f32": F32, "bf16": BF16}[mm_dtype]
    zn_dt = BF16 if mm_dtype == "bf16" else F32

    with ExitStack() as ctx:
        tc = ctx.enter_context(tile.TileContext(nc))
        singles = ctx.enter_context(tc.tile_pool(name="singles", bufs=1))
        zbuf = ctx.enter_context(tc.tile_pool(name="zbuf", bufs=1))
        znbuf = ctx.enter_context(tc.tile_pool(name="znbuf", bufs=1))
        scr = ctx.enter_context(tc.tile_pool(name="scr", bufs=2))
        psum = ctx.enter_context(tc.tile_pool(name="psum", bufs=2, space="PSUM"))

        ident_g = singles.tile([128, 128], zn_dt)
        make_identity(nc, ident_g)
        # DVE-copy so consumers of the identity depend on DVE, not Pool.
        ident = singles.tile([128, 128], zn_dt)
        nc.vector.tensor_copy(ident, ident_g)

        # -LSE_SHIFT bias, produced on ACT itself (activations then only ever
        # wait on PE).
        neg_shift = singles.tile([128, 1], F32)
        one_ap = nc.const_aps.tensor(1.0, (128, 1))
        nc.scalar.mul(neg_shift, one_ap, -LSE_SHIFT)

        # Dummy weight tile for PE wait-splitter ldweights.
        ldw_dummy = singles.tile([1, 1], BF16)
        nc.vector.memset(ldw_dummy, 0.0)

        znT = singles.tile([128, M2], mm_dt)         # [D, M2] rhs columns
        znT_slab = singles.tile([128, ROWS], mm_dt)  # [D, ROWS] lhsT
        z_sp = singles.tile([128, 2 * NT_SP, D], F32)
        zn_sp = singles.tile([128, 2 * NT_SP, D], zn_dt)
        nrm2 = singles.tile([128, 2 * NT_SP + 64], F32)
        lgn = singles.tile([128, 2 * NT_SP + 64], F32)
        inv = singles.tile([128, 2 * NT_SP + 64], F32)
        praw = singles.tile([128, NT_SP], F32)
        draw = singles.tile([128, NT_SP], F32)
        sacc = singles.tile([128, MI, NGROUPS], F32)
        eo_sink = singles.tile([128, MI * NGROUPS], F32)

        # PE wait-splitter: a real PE instruction (1x1 ldweights — harmless,
        # every matmul self-loads its weights) that absorbs one cross-engine
        # wait via an explicit sync dep.
        def pe_absorb(dep):
            lw = nc.tensor.ldweights(weights=ldw_dummy)
            add_dep_helper(lw.ins, dep.ins, sync=True,
                           reason="absorb cross-engine wait on PE")

        # psum slot bookkeeping: reader instruction of each allocated tile,
        # so slot reuse (bufs=2 -> two tiles back) can be absorbed on PE.
        readers = []

        def new_ps(dtype):
            if len(readers) >= 2:
                pe_absorb(readers[-2])
            return psum.tile([128, GW], dtype, tag="ps", name="ps")

        # ---------- slab phase ----------
        nc.sync.dma_start(
            out=z_sp[:, 0:NT_SP, :],
            in_=z_slab[:, :].rearrange("(t p) d -> p t d", p=128),
        )
        nc.sync.dma_start(
            out=z_sp[:, NT_SP:2 * NT_SP, :],
            in_=z_part[:, :].rearrange("(t p) d -> p t d", p=128),
        )
        # normalize a half (slab or partner) of z_sp: norms, inv, scale
        def process_sp_half(h):
            sq = scr.tile([128, NT_SP, D], F32, tag="sq", name="sq")
            nc.vector.tensor_mul(
                sq, z_sp[:, h * NT_SP:(h + 1) * NT_SP, :],
                z_sp[:, h * NT_SP:(h + 1) * NT_SP, :],
            )
            nc.vector.tensor_reduce(
                out=nrm2[:, h * NT_SP:(h + 1) * NT_SP], in_=sq,
                axis=mybir.AxisListType.X, op=ALU.add,
            )
            sl = slice(h * NT_SP, (h + 1) * NT_SP)
            nc.vector.tensor_scalar_max(nrm2[:, sl], nrm2[:, sl], 1e-16)
            nc.scalar.activation(out=lgn[:, sl], in_=nrm2[:, sl], func=AF.Ln)
            nc.scalar.activation(out=inv[:, sl], in_=lgn[:, sl],
                                 func=AF.Exp, scale=-0.5)
            iv = inv[:, sl]
            iv_b = bass.AP(tensor=iv.tensor, offset=iv.offset,
                           ap=[iv.ap[0], iv.ap[1], [0, D]])
            return nc.vector.scalar_tensor_tensor(
                out=zn_sp[:, sl, :], in0=z_sp[:, sl, :], scalar=0.0, in1=iv_b,
                op0=ALU.bypass, op1=ALU.mult,
            )

        # slab half only — the partner half is deferred past the pipeline
        # lead-in (its results feed only the epilogue)
        last_ts_sp = process_sp_half(0)
        # slab transposes -> znT_slab (pe_absorb covers the DVE data ticks;
        # the diagonal/positive dots are deferred past the main loop to keep
        # the pipeline lead-in short)
        ps = new_ps(zn_dt)
        pe_absorb(last_ts_sp)
        for u in range(NT_SP):
            nc.tensor.transpose(out=ps[:, u * 128:(u + 1) * 128],
                                in_=zn_sp[:, u, :], identity=ident)
        cp = nc.vector.tensor_copy(out=znT_slab, in_=ps[:, 0:ROWS])
        readers.append(cp)

        # ---------- pipelined column groups ----------
        z_re = z_full[:, :].rearrange("(t p) d -> p t d", p=128)
        gidx = 0
        tile_base = 0
        for g, ntg in enumerate(GROUP_TILES):
            gw = ntg * 128
            cb = tile_base * 128          # column base in znT
            co = 2 * NT_SP + tile_base    # offset into nrm2/lgn/inv
            zg = zbuf.tile([128, ntg, D], F32, tag=f"zg{g}", name="zg")
            nc.sync.dma_start(out=zg,
                              in_=z_re[:, tile_base:tile_base + ntg, :])
            sqg = scr.tile([128, ntg, D], F32, tag="sqg", name="sqg")
            nc.vector.tensor_mul(sqg, zg, zg)
            nc.vector.tensor_reduce(out=nrm2[:, co:co + ntg], in_=sqg,
                                    axis=mybir.AxisListType.X, op=ALU.add)
            nc.vector.tensor_scalar_max(
                nrm2[:, co:co + ntg], nrm2[:, co:co + ntg], 1e-16
            )
            nc.scalar.activation(out=lgn[:, co:co + ntg],
                                 in_=nrm2[:, co:co + ntg], func=AF.Ln)
            nc.scalar.activation(out=inv[:, co:co + ntg],
                                 in_=lgn[:, co:co + ntg], func=AF.Exp,
                                 scale=-0.5)
            zng = znbuf.tile([128, ntg, D], zn_dt, tag=f"zng{g}", name="zng")
            iv = inv[:, co:co + ntg]
            iv_b = bass.AP(tensor=iv.tensor, offset=iv.offset,
                           ap=[iv.ap[0], iv.ap[1], [0, D]])
            last_ts = nc.vector.scalar_tensor_tensor(
                out=zng, in0=zg, scalar=0.0, in1=iv_b,
                op0=ALU.bypass, op1=ALU.mult,
            )

            # transpose group (pe_absorb covers the fresh DVE data ticks)
            ps = new_ps(zn_dt)
            pe_absorb(last_ts)
            for u in range(ntg):
                nc.tensor.transpose(out=ps[:, u * 128:(u + 1) * 128],
                                    in_=zng[:, u, :], identity=ident)
            cp = nc.vector.tensor_copy(out=znT[:, cb:cb + gw],
                                       in_=ps[:, 0:gw])
            readers.append(cp)

            # M-blocks for this column group
            for mi in range(MI):
                psm = new_ps(F32)
                if mi == 0:
                    # first user of this group's znT columns: absorb the DVE
                    # copy tick on PE
                    pe_absorb(cp)
                lhsT = znT_slab[:, mi * 128:(mi + 1) * 128]
                for k in range((gw + 511) // 512):
                    w = min(512, gw - k * 512)
                    nc.tensor.matmul(
                        out=psm[:, k * 512:k * 512 + w],
                        lhsT=lhsT,
                        rhs=znT[:, cb + k * 512:cb + k * 512 + w],
                        start=True, stop=True,
                    )
                act = nc.scalar.activation(
                    out=eo_sink[:, gidx:gidx + 1].broadcast_to((128, gw)),
                    in_=psm[:, 0:gw], func=AF.Exp,
                    scale=TEMP_INV, bias=neg_shift,
                    accum_out=sacc[:, mi, g:g + 1],
                )
                readers.append(act)
                gidx += 1

            if g == 1:
                # partner half of the slab (feeds only the epilogue) and the
                # diagonal/positive dots, scheduled mid-loop where DVE idles
                process_sp_half(1)
                sqd = scr.tile([128, NT_SP, D], F32, tag="sq2")
                nc.vector.tensor_mul(sqd, zn_sp[:, 0:NT_SP, :],
                                     zn_sp[:, 0:NT_SP, :])
                nc.vector.tensor_reduce(out=draw, in_=sqd,
                                        axis=mybir.AxisListType.X, op=ALU.add)
                sqp = scr.tile([128, NT_SP, D], F32, tag="sq2")
                nc.vector.tensor_mul(sqp, zn_sp[:, 0:NT_SP, :],
                                     zn_sp[:, NT_SP:2 * NT_SP, :])
                nc.vector.tensor_reduce(out=praw, in_=sqp,
                                        axis=mybir.AxisListType.X, op=ALU.add)
            tile_base += ntg

        # ---------- epilogue ----------
        s_all = singles.tile([128, MI], F32)
        nc.vector.tensor_reduce(
            out=s_all, in_=sacc, axis=mybir.AxisListType.X, op=ALU.add
        )
        dexp = singles.tile([128, MI], F32)
        nc.scalar.activation(out=dexp, in_=draw, func=AF.Exp,
                             scale=TEMP_INV, bias=neg_shift)
        pexp = singles.tile([128, MI], F32)
        nc.scalar.activation(out=pexp, in_=praw, func=AF.Exp,
                             scale=2.0 * TEMP_INV, bias=neg_shift)
        den = singles.tile([128, MI], F32)
        nc.vector.tensor_sub(den, s_all, dexp)
        nc.vector.tensor_add(den, den, pexp)
        lg = singles.tile([128, MI], F32)
        nc.scalar.activation(out=lg, in_=den, func=AF.Ln)
        pos = singles.tile([128, MI], F32)
        nc.vector.tensor_scalar_mul(pos, praw, 2.0 * TEMP_INV)
        lt = singles.tile([128, MI], F32)
        nc.vector.tensor_sub(lt, lg, pos)
        lout = singles.tile([128, MI], F32)
        nc.vector.tensor_scalar_add(lout, lt, LSE_SHIFT)
        nc.gpsimd.dma_start(out=out_l[:, :], in_=lout)

    return nc


_NC_CACHE: dict = {}


def _get_nc(mm_dtype: str = "bf16") -> bass.Bass:
    if mm_dtype not in _NC_CACHE:
        _NC_CACHE[mm_dtype] = build_kernel(mm_dtype)
    return _NC_CACHE[mm_dtype]


def make_in_maps(z1: np.ndarray, z2: np.ndarray):
    z = np.ascontiguousarray(
        np.concatenate([z1, z2], axis=0), dtype=np.float32
    )
    in_maps = []
    for c in range(N_CORES):
        lo = c * ROWS
        plo = (lo + N) % M2
        in_maps.append({
            "z_full": z,
            "z_slab": np.ascontiguousarray(z[lo:lo + ROWS]),
            "z_part": np.ascontiguousarray(z[plo:plo + ROWS]),
        })
    return in_maps


def finish(results) -> np.ndarray:
    total = 0.0
    for r in results:
        total += r["out_l"].astype(np.float64).sum()
    return np.float32(total / (float(M2) * float(M2)))


def kernel(z1: np.ndarray, z2: np.ndarray, mm_dtype: str = "bf16",
           **run_kwargs) -> np.ndarray:
    nc = _get_nc(mm_dtype)
    in_maps = make_in_maps(z1, z2)
    res = run_bass_kernel_spmd(nc, in_maps, core_ids=list(range(N_CORES)), **run_kwargs)
    out = finish(res.results)
    kernel.last_results = res
    return out


# revision 24
# speedup vs baseline: 1.0261x; 1.0261x over previous
# Contrastive (NT-Xent / SimCLR) loss kernel for Trainium2, 8 NeuronCores.
#
# Reference computation (N=4096, D=128, T=0.1, M=2N=8192):
#   z  = concat(z1, z2)                      [M, D]
#   zn = z / max(||z||, 1e-8)                row-normalized
#   sim = (zn @ zn.T) / T                    [M, M]
#   pos_r = sim[r, partner(r)] + sim[partner(r), r] = 2*sim[r, partner(r)]
#   loss = mean_r( LSE(logits_r) - pos_r ) / M
#     where logits_r = [pos_r] ++ {sim[r, j] : j != r}
#
# Per-row algebra used on device (constant shift m = 1/T = 10):
#   S_all_r = sum_j exp(sim[r, j] - 10)                 (all M columns)
#   dexp_r  = exp(sim[r, r] - 10)                       (diagonal, excluded)
#   pexp_r  = exp(pos_r - 10)
#   L_r     = 10 + log(pexp_r + S_all_r - dexp_r) - pos_r
#   loss    = sum_r L_r / M^2
#
# Sharding: rows of z split across 8 cores (1024 rows/core). Every core
# receives the full z (for the all-gathered rhs), plus its own row slab and
# the partner slab (rows +-N) so the diagonal/positive terms are computed
# locally without any cross-core traffic. Host sums 8 partial [128, 8] L
# tiles -> scalar loss.
#
# Per-core pipeline (v2 — column-group pipelined):
#   Slab phase: load + normalize the slab and partner rows, PE-transpose the
#   slab into znT_slab [D, 1024] (fp32r lhsT), take diagonal/positive row
#   dots on DVE.
#   Then 4 column groups of 2048 (16 row tiles each), pipelined across
#   DMA / DVE / PE / ACT:
#     load z rows -> row norms (DVE mul+reduce) -> inv = exp(-.5 ln(nrm2))
#     (ACT) -> normalize (DVE) -> PE-transpose into a [128, 2048] PSUM tile
#     -> DVE copy into znT columns (rounds to fp32r) -> 8 M-blocks of
#     4 fp32r matmuls [128x512] + one ACT exp(10G-10) with accum_out row-sum.
#   Epilogue combines S_all with the diagonal/positive terms, one log, and
#   DMAs the [128, 8] per-row loss tile out.
#
# This toolchain's walrus rejects any instruction carrying more than ONE sync
# wait ("Too many sync wait commands"), which shapes several oddities here:
#   - sacrificial 1x1 `ldweights` instructions absorb cross-engine waits so
#     matmuls keep a single wait (bacc fuses NoOps, so a real PE instruction
#     is required);
#   - each transpose group starts with a dummy transpose that reads the
#     last-normalized tile (absorbs the DVE data wait);
#   - activation outputs go through disjoint stride-0 broadcast APs onto a
#     sink tile (only accum_out matters), avoiding WAW waits entirely;
#   - InstTensorTensorReduce fails codegen outright -> mul + tensor_reduce;
#   - the Tile kernel-tail drain is re-emitted as one single-wait drain per
#     proc (see _split_drain_and_barrier);
#   - the result DMA uses gpsimd SWDGE so it does not share a HWDGE queue
#     with the input loads.

import numpy as np

import concourse.bass as bass
import concourse.mybir as mybir
import concourse.tile as tile
from concourse.tile import add_dep_helper
from contextlib import ExitStack

from concourse.bass_utils import run_bass_kernel_spmd
from concourse.masks import make_identity
from concourse.vector_clock import ScopedClock, VectorClock


def _split_drain_and_barrier(self, tick_clock, wait_clock):
    """Replacement for TileContext._drain_and_barrier: the stock version
    emits ONE drain carrying a wait for every live proc (13+ here), which this
    walrus build rejects ("Too many sync wait commands"). Emit one single-wait
    drain per proc instead, then the normal barrier/cleanup."""
    nc = self.nc
    ticks = list(tick_clock.global_clock)
    for proc, t in enumerate(ticks):
        if t <= 0:
            continue
        d = nc.sync.drain()
        single = VectorClock()
        single.require_at_least(proc, t)
        wait_clock.add_sem_waits(d.ins, ScopedClock({None: single}))
    nc.all_engine_barrier()
    assert self.sems is not None
    popped = nc._tile_sem_poison_stack.pop()
    assert popped is self._sem_poison
    nc.clear_and_free_semaphores(list(self.sems.allocated().values()))
    nc.all_engine_barrier()


tile.TileContext._drain_and_barrier = _split_drain_and_barrier

F32 = mybir.dt.float32
F32R = mybir.dt.float32r
BF16 = mybir.dt.bfloat16
AF = mybir.ActivationFunctionType
ALU = mybir.AluOpType

N_CORES = 8
N = 4096
D = 128
M2 = 2 * N                 # 8192 rows total
ROWS = M2 // N_CORES       # 1024 rows per core
NT_SP = ROWS // 128        # 8 row tiles per slab
MI = ROWS // 128           # 8 M-chunks per core
CGROUPS = 4                # column groups
NTG = 16                   # row tiles per column group
GW = NTG * 128             # 2048 columns per group
NMM = GW // 512            # matmuls per M-block

TEMP_INV = 10.0            # 1/T
LSE_SHIFT = 10.0           # constant max-shift for the log-sum-exp


def build_kernel(mm_dtype: str = "bf16") -> bass.Bass:
    nc = bass.Bass()

    z_full = nc.dram_tensor("z_full", [M2, D], F32, kind="ExternalInput")
    z_slab = nc.dram_tensor("z_slab", [ROWS, D], F32, kind="ExternalInput")
    z_part = nc.dram_tensor("z_part", [ROWS, D], F32, kind="ExternalInput")
    out_l = nc.dram_tensor("out_l", [128, MI], F32, kind="ExternalOutput")

    mm_dt = {"f32r": F32R, "f32": F32, "bf16": BF16}[mm_dtype]
    zn_dt = BF16 if mm_dtype == "bf16" else F32

    with ExitStack() as ctx:
        tc = ctx.enter_context(tile.TileContext(nc))
        singles = ctx.enter_context(tc.tile_pool(name="singles", bufs=1))
        zbuf = ctx.enter_context(tc.tile_pool(name="zbuf", bufs=4))
        znbuf = ctx.enter_context(tc.tile_pool(name="znbuf", bufs=4))
        scr = ctx.enter_context(tc.tile_pool(name="scr", bufs=2))
        psum = ctx.enter_context(tc.tile_pool(name="psum", bufs=2, space="PSUM"))

        ident_g = singles.tile([128, 128], zn_dt)
        make_identity(nc, ident_g)
        # DVE-copy so consumers of the identity depend on DVE, not Pool.
        ident = singles.tile([128, 128], zn_dt)
        nc.vector.tensor_copy(ident, ident_g)

        # -LSE_SHIFT bias, produced on ACT itself (activations then only ever
        # wait on PE).
        neg_shift = singles.tile([128, 1], F32)
        one_ap = nc.const_aps.tensor(1.0, (128, 1))
        nc.scalar.mul(neg_shift, one_ap, -LSE_SHIFT)

        # Dummy weight tile for PE wait-splitter ldweights.
        ldw_dummy = singles.tile([1, 1], BF16)
        nc.vector.memset(ldw_dummy, 0.0)

        znT = singles.tile([128, M2], mm_dt)         # [D, M2] rhs columns
        znT_slab = singles.tile([128, ROWS], mm_dt)  # [D, ROWS] lhsT
        z_sp = singles.tile([128, 2 * NT_SP, D], F32)
        zn_sp = singles.tile([128, 2 * NT_SP, D], zn_dt)
        nrm2 = singles.tile([128, 2 * NT_SP + CGROUPS * NTG], F32)
        lgn = singles.tile([128, 2 * NT_SP + CGROUPS * NTG], F32)
        inv = singles.tile([128, 2 * NT_SP + CGROUPS * NTG], F32)
        praw = singles.tile([128, NT_SP], F32)
        draw = singles.tile([128, NT_SP], F32)
        sacc = singles.tile([128, MI, CGROUPS], F32)
        eo_sink = singles.tile([128, MI * CGROUPS], F32)

        # PE wait-splitter: a real PE instruction (1x1 ldweights — harmless,
        # every matmul self-loads its weights) that absorbs one cross-engine
        # wait via an explicit sync dep.
        def pe_absorb(dep):
            lw = nc.tensor.ldweights(weights=ldw_dummy)
            add_dep_helper(lw.ins, dep.ins, sync=True,
                           reason="absorb cross-engine wait on PE")

        # psum slot bookkeeping: reader instruction of each allocated tile,
        # so slot reuse (bufs=2 -> two tiles back) can be absorbed on PE.
        readers = []

        def new_ps(dtype):
            if len(readers) >= 2:
                pe_absorb(readers[-2])
            return psum.tile([128, GW], dtype, tag="ps", name="ps")

        # ---------- slab phase ----------
        nc.sync.dma_start(
            out=z_sp[:, 0:NT_SP, :],
            in_=z_slab[:, :].rearrange("(t p) d -> p t d", p=128),
        )
        nc.sync.dma_start(
            out=z_sp[:, NT_SP:2 * NT_SP, :],
            in_=z_part[:, :].rearrange("(t p) d -> p t d", p=128),
        )
        # one mul+reduce per DMA half so each op waits on a single queue
        for h in range(2):
            sq = scr.tile([128, NT_SP, D], F32, tag="sq", name="sq")
            nc.vector.tensor_mul(
                sq, z_sp[:, h * NT_SP:(h + 1) * NT_SP, :],
                z_sp[:, h * NT_SP:(h + 1) * NT_SP, :],
            )
            nc.vector.tensor_reduce(
                out=nrm2[:, h * NT_SP:(h + 1) * NT_SP], in_=sq,
                axis=mybir.AxisListType.X, op=ALU.add,
            )
        nc.vector.tensor_scalar_max(
            nrm2[:, 0:2 * NT_SP], nrm2[:, 0:2 * NT_SP], 1e-16
        )
        nc.scalar.activation(out=lgn[:, 0:2 * NT_SP], in_=nrm2[:, 0:2 * NT_SP],
                             func=AF.Ln)
        nc.scalar.activation(out=inv[:, 0:2 * NT_SP], in_=lgn[:, 0:2 * NT_SP],
                             func=AF.Exp, scale=-0.5)
        iv = inv[:, 0:2 * NT_SP]
        iv_b = bass.AP(tensor=iv.tensor, offset=iv.offset,
                       ap=[iv.ap[0], iv.ap[1], [0, D]])
        last_ts_sp = nc.vector.scalar_tensor_tensor(
            out=zn_sp, in0=z_sp, scalar=0.0, in1=iv_b,
            op0=ALU.bypass, op1=ALU.mult,
        )
        # slab transposes -> znT_slab (pe_absorb covers the DVE data ticks;
        # the diagonal/positive dots are deferred past the main loop to keep
        # the pipeline lead-in short)
        ps = new_ps(zn_dt)
        pe_absorb(last_ts_sp)
        for u in range(NT_SP):
            nc.tensor.transpose(out=ps[:, u * 128:(u + 1) * 128],
                                in_=zn_sp[:, u, :], identity=ident)
        cp = nc.vector.tensor_copy(out=znT_slab, in_=ps[:, 0:ROWS])
        readers.append(cp)

        # ---------- pipelined column groups ----------
        z_re = z_full[:, :].rearrange("(t p) d -> p t d", p=128)
        gidx = 0
        for g in range(CGROUPS):
            co = 2 * NT_SP + g * NTG   # column offset into nrm2/lgn/inv
            zg = zbuf.tile([128, NTG, D], F32, tag="zg")
            nc.sync.dma_start(out=zg, in_=z_re[:, g * NTG:(g + 1) * NTG, :])
            sqg = scr.tile([128, NTG, D], F32, tag="sqg")
            nc.vector.tensor_mul(sqg, zg, zg)
            nc.vector.tensor_reduce(out=nrm2[:, co:co + NTG], in_=sqg,
                                    axis=mybir.AxisListType.X, op=ALU.add)
            nc.vector.tensor_scalar_max(
                nrm2[:, co:co + NTG], nrm2[:, co:co + NTG], 1e-16
            )
            nc.scalar.activation(out=lgn[:, co:co + NTG],
                                 in_=nrm2[:, co:co + NTG], func=AF.Ln)
            nc.scalar.activation(out=inv[:, co:co + NTG],
                                 in_=lgn[:, co:co + NTG], func=AF.Exp,
                                 scale=-0.5)
            zng = znbuf.tile([128, NTG, D], zn_dt, tag="zng")
            iv = inv[:, co:co + NTG]
            iv_b = bass.AP(tensor=iv.tensor, offset=iv.offset,
                           ap=[iv.ap[0], iv.ap[1], [0, D]])
            last_ts = nc.vector.scalar_tensor_tensor(
                out=zng, in0=zg, scalar=0.0, in1=iv_b,
                op0=ALU.bypass, op1=ALU.mult,
            )

            # transpose group (pe_absorb covers the fresh DVE data ticks)
            ps = new_ps(zn_dt)
            pe_absorb(last_ts)
            for u in range(NTG):
                nc.tensor.transpose(out=ps[:, u * 128:(u + 1) * 128],
                                    in_=zng[:, u, :], identity=ident)
            cp = nc.vector.tensor_copy(out=znT[:, g * GW:(g + 1) * GW], in_=ps)
            readers.append(cp)

            # M-blocks for this column group
            for mi in range(MI):
                psm = new_ps(F32)
                if mi == 0:
                    # first user of this group's znT columns: absorb the DVE
                    # copy tick on PE
                    pe_absorb(cp)
                lhsT = znT_slab[:, mi * 128:(mi + 1) * 128]
                for k in range(NMM):
                    ni = g * NMM + k
                    nc.tensor.matmul(
                        out=psm[:, k * 512:(k + 1) * 512],
                        lhsT=lhsT,
                        rhs=znT[:, ni * 512:(ni + 1) * 512],
                        start=True, stop=True,
                    )
                act = nc.scalar.activation(
                    out=eo_sink[:, gidx:gidx + 1].broadcast_to((128, GW)),
                    in_=psm, func=AF.Exp,
                    scale=TEMP_INV, bias=neg_shift,
                    accum_out=sacc[:, mi, g:g + 1],
                )
                readers.append(act)
                gidx += 1

            if g == 1:
                # diagonal & positive raw dots, scheduled mid-loop where DVE
                # has slack (results only needed by the epilogue)
                sqd = scr.tile([128, NT_SP, D], F32, tag="sq2")
                nc.vector.tensor_mul(sqd, zn_sp[:, 0:NT_SP, :],
                                     zn_sp[:, 0:NT_SP, :])
                nc.vector.tensor_reduce(out=draw, in_=sqd,
                                        axis=mybir.AxisListType.X, op=ALU.add)
                sqp = scr.tile([128, NT_SP, D], F32, tag="sq2")
                nc.vector.tensor_mul(sqp, zn_sp[:, 0:NT_SP, :],
                                     zn_sp[:, NT_SP:2 * NT_SP, :])
                nc.vector.tensor_reduce(out=praw, in_=sqp,
                                        axis=mybir.AxisListType.X, op=ALU.add)

        # ---------- epilogue ----------
        s_all = singles.tile([128, MI], F32)
        nc.vector.tensor_reduce(
            out=s_all, in_=sacc, axis=mybir.AxisListType.X, op=ALU.add
        )
        dexp = singles.tile([128, MI], F32)
        nc.scalar.activation(out=dexp, in_=draw, func=AF.Exp,
                             scale=TEMP_INV, bias=neg_shift)
        pexp = singles.tile([128, MI], F32)
        nc.scalar.activation(out=pexp, in_=praw, func=AF.Exp,
                             scale=2.0 * TEMP_INV, bias=neg_shift)
        den = singles.tile([128, MI], F32)
        nc.vector.tensor_sub(den, s_all, dexp)
        nc.vector.tensor_add(den, den, pexp)
        lg = singles.tile([128, MI], F32)
        nc.scalar.activation(out=lg, in_=den, func=AF.Ln)
        pos = singles.tile([128, MI], F32)
        nc.vector.tensor_scalar_mul(pos, praw, 2.0 * TEMP_INV)
        lt = singles.tile([128, MI], F32)
        nc.vector.tensor_sub(lt, lg, pos)
        lout = singles.tile([128, MI], F32)
        nc.vector.tensor_scalar_add(lout, lt, LSE_SHIFT)
        nc.gpsimd.dma_start(out=out_l[:, :], in_=lout)

    return nc


_NC_CACHE: dict = {}


def _get_nc(mm_dtype: str = "bf16") -> bass.Bass:
    if mm_dtype not in _NC_CACHE:
        _NC_CACHE[mm_dtype] = build_kernel(mm_dtype)
    return _NC_CACHE[mm_dtype]


def make_in_maps(z1: np.ndarray, z2: np.ndarray):
    z = np.ascontiguousarray(
        np.concatenate([z1, z2], axis=0), dtype=np.float32
    )
    in_maps = []
    for c in range(N_CORES):
        lo = c * ROWS
        plo = (lo + N) % M2
        in_maps.append({
            "z_full": z,
            "z_slab": np.ascontiguousarray(z[lo:lo + ROWS]),
            "z_part": np.ascontiguousarray(z[plo:plo + ROWS]),
        })
    return in_maps


def finish(results) -> np.ndarray:
    total = 0.0
    for r in results:
        total += r["out_l"].astype(np.float64).sum()
    return np.float32(total / (float(M2) * float(M2)))


def kernel(z1: np.ndarray, z2: np.ndarray, mm_dtype: str = "bf16",
           **run_kwargs) -> np.ndarray:
    nc = _get_nc(mm_dtype)
    in_maps = make_in_maps(z1, z2)
    res = run_bass_kernel_spmd(nc, in_maps, core_ids=list(range(N_CORES)), **run_kwargs)
    out = finish(res.results)
    kernel.last_results = res
    return out
